# revision 6
# baseline (speedup 1.0000x reference)
"""Trainium2 Bass kernel for nn_CharRNN: bidirectional char-GRU + temporal max-pool.

Problem shapes (hardcoded): B=64, S=256, T=16, V=262, E=64, H=32.
16384 independent char sequences ("words") are sharded 8 ways (2048 words/core).

Per-core layout ("dir-major, 2-group"): every [128, F] tile's partition axis is
split into 4 blocks of 32: [f-dir group0 | f-dir group1 | b-dir group0 | b-dir group1]
where group0 = words 0..1023 and group1 = words 1024..2047 of the core's slice,
and the free axis is the word-within-group. E-carrying tiles (embedded chars) use
2 blocks of 64: [E dims of group0 words | E dims of group1 words].

Per GRU step s (f consumes char s, b consumes char 15-s):
  psum_R = Wih_r_f·e_f + Wih_r_b·e_b + Whh_r·h      (3 accumulating matmuls)
  psum_Z = likewise
  psum_M = Wih_n_f·e_f + Wih_n_b·e_b                 (input-gate n part)
  psum_N = Whh_n·h                                   (hidden n part)
  r = sigmoid(psum_R + bias_r)   [ACT, per-partition bias]
  z = sigmoid(psum_Z + bias_z)
  hn = (psum_N + bhh_n) * r      [DVE scalar_tensor_tensor]
  npre = hn + psum_M
  n = tanh(npre + bih_n)
  h' = n + z*(h - n)
  ymax = max(ymax, h')

The char embedding lookup is a GPSIMD ap_gather from a replicated emb.T table.
"""

import sys
import os

sys.path.insert(0, "/opt/trn_rl_repo")

import numpy as np

import concourse.bacc as bacc
import concourse.tile as tile
from concourse import mybir
from concourse.bass_utils import run_bass_kernel_spmd
from concourse.alu_op_type import AluOpType as Alu

B, S, T = 64, 256, 16
V, E, H = 262, 64, 32
NCORES = 8
WPC = 16384 // NCORES  # words per core = 2048
G = WPC // 2  # words per group = 1024
FH = G // 2  # free-dim half-chunk = 512

F32 = mybir.dt.float32
I16 = mybir.dt.int16

AF = mybir.ActivationFunctionType

_CACHE = {}


def _build_program():
    nc = bacc.Bacc("TRN2", target_bir_lowering=False, debug=False, num_devices=NCORES)

    # DRAM I/O
    d_idx = nc.dram_tensor("idx", [128, T, G // 16], I16, kind="ExternalInput").ap()
    d_embT = nc.dram_tensor("embT", [128, V], F32, kind="ExternalInput").ap()
    ih_names = ["ihR_f", "ihR_b", "ihZ_f", "ihZ_b", "ihN_f", "ihN_b"]
    hh_names = ["hhR", "hhZ", "hhN"]
    d_w = {
        n: nc.dram_tensor(n, [128, 128], F32, kind="ExternalInput").ap()
        for n in ih_names + hh_names
    }
    bias_names = ["biasR", "biasZ", "biasN", "bhhN"]
    d_b = {
        n: nc.dram_tensor(n, [128, 1], F32, kind="ExternalInput").ap()
        for n in bias_names
    }
    d_out = nc.dram_tensor("out", [128, WPC // 2], F32, kind="ExternalOutput").ap()

    with tile.TileContext(nc) as tc:
        with (
            tc.tile_pool(name="consts", bufs=1) as consts,
            tc.tile_pool(name="xe", bufs=1) as xepool,
            tc.tile_pool(name="state", bufs=1) as state,
            tc.tile_pool(name="work", bufs=4) as work,
            tc.tile_pool(name="ps", bufs=8, space="PSUM") as ps,
        ):
            # --- load constants ---
            s_idx = consts.tile([128, T, G // 16], I16)
            nc.sync.dma_start(out=s_idx, in_=d_idx)
            s_embT = consts.tile([128, V], F32)
            nc.sync.dma_start(out=s_embT, in_=d_embT)
            s_w = {}
            for n in ih_names + hh_names:
                s_w[n] = consts.tile([128, 128], F32, name=n)
                nc.sync.dma_start(out=s_w[n], in_=d_w[n])
            s_b = {}
            for n in bias_names:
                s_b[n] = consts.tile([128, 1], F32, name=n)
                nc.sync.dma_start(out=s_b[n], in_=d_b[n])

            # --- gather char embeddings: xe[:, t, :] = embT[:, x[word, t]] ---
            # partitions 0:64 gather group0 words, 64:128 gather group1 words
            xe = xepool.tile([128, T, G], F32)
            embT3 = s_embT.rearrange("p (n d) -> p n d", d=1)
            order = []
            for i in range(T // 2):
                order += [i, T - 1 - i]
            for t in order:
                nc.gpsimd.ap_gather(
                    out_ap=xe[:, t, :].rearrange("p (n d) -> p n d", d=1),
                    in_ap=embT3,
                    idxs_ap=s_idx[:, t, :],
                    channels=128,
                    num_elems=V,
                    d=1,
                    num_idxs=G,
                )

            # --- state tiles ---
            h = state.tile([128, G], F32)
            nc.vector.memset(h, 0.0)
            ymax = state.tile([128, G], F32)
            nc.vector.memset(ymax, -3.0e38)

            hprev = h
            for s in range(T):
                tf, tb = s, T - 1 - s
                hnew = work.tile([128, G], F32, tag="h", bufs=2)
                for c in range(2):
                    sl = slice(c * FH, (c + 1) * FH)
                    xf = xe[:, tf, sl]
                    xb = xe[:, tb, sl]
                    pR = ps.tile([128, FH], F32, tag="ps")
                    pZ = ps.tile([128, FH], F32, tag="ps")
                    pM = ps.tile([128, FH], F32, tag="ps")
                    pN = ps.tile([128, FH], F32, tag="ps")
                    nc.tensor.matmul(pR, lhsT=s_w["ihR_f"], rhs=xf, start=True, stop=False)
                    nc.tensor.matmul(pR, lhsT=s_w["ihR_b"], rhs=xb, start=False, stop=False)
                    nc.tensor.matmul(pR, lhsT=s_w["hhR"], rhs=hprev[:, sl], start=False, stop=True)
                    nc.tensor.matmul(pZ, lhsT=s_w["ihZ_f"], rhs=xf, start=True, stop=False)
                    nc.tensor.matmul(pZ, lhsT=s_w["ihZ_b"], rhs=xb, start=False, stop=False)
                    nc.tensor.matmul(pZ, lhsT=s_w["hhZ"], rhs=hprev[:, sl], start=False, stop=True)
                    nc.tensor.matmul(pM, lhsT=s_w["ihN_f"], rhs=xf, start=True, stop=False)
                    nc.tensor.matmul(pM, lhsT=s_w["ihN_b"], rhs=xb, start=False, stop=True)
                    nc.tensor.matmul(pN, lhsT=s_w["hhN"], rhs=hprev[:, sl], start=True, stop=True)

                    r = work.tile([128, FH], F32, tag="r")
                    z = work.tile([128, FH], F32, tag="z")
                    nc.scalar.activation(r, pR, AF.Sigmoid, bias=s_b["biasR"])
                    nc.scalar.activation(z, pZ, AF.Sigmoid, bias=s_b["biasZ"])
                    hn = work.tile([128, FH], F32, tag="hn")
                    nc.vector.scalar_tensor_tensor(
                        out=hn, in0=pN, scalar=s_b["bhhN"], in1=r,
                        op0=Alu.add, op1=Alu.mult,
                    )
                    npre = work.tile([128, FH], F32, tag="npre")
                    nc.vector.tensor_tensor(out=npre, in0=hn, in1=pM, op=Alu.add)
                    n = work.tile([128, FH], F32, tag="n")
                    nc.scalar.activation(n, npre, AF.Tanh, bias=s_b["biasN"])
                    d = work.tile([128, FH], F32, tag="d")
                    nc.vector.tensor_tensor(out=d, in0=hprev[:, sl], in1=n, op=Alu.subtract)
                    e = work.tile([128, FH], F32, tag="e")
                    nc.vector.tensor_tensor(out=e, in0=z, in1=d, op=Alu.mult)
                    nc.vector.tensor_tensor(out=hnew[:, sl], in0=n, in1=e, op=Alu.add)
                    nc.vector.tensor_tensor(
                        out=ymax[:, sl], in0=ymax[:, sl], in1=hnew[:, sl], op=Alu.max
                    )
                hprev = hnew

            nc.sync.dma_start(out=d_out, in_=ymax)

    nc.compile()
    return nc


def _prep_inputs(x, emb, Wih_f, Whh_f, bih_f, bhh_f, Wih_b, Whh_b, bih_b, bhh_b):
    """Host-side weight/index layout prep (weight-space transforms + sharding only)."""
    f32 = np.float32
    x_flat = np.asarray(x).reshape(16384, T).astype(np.int16)

    embT2 = np.concatenate([np.asarray(emb, f32).T, np.asarray(emb, f32).T], axis=0)
    embT2 = np.ascontiguousarray(embT2)  # [128, V]

    def ih_tile(W, gate, dir_b):
        # W: [96, E]; gate 0=r,1=z,2=n. M-cols: f at 0:64, b at 64:128 (2 groups of 32).
        L = np.zeros((128, 128), f32)
        Wg = np.asarray(W, f32)[gate * H:(gate + 1) * H, :]  # [32, E]
        off = 64 if dir_b else 0
        L[0:64, off + 0:off + 32] = Wg.T
        L[64:128, off + 32:off + 64] = Wg.T
        return L

    def hh_tile(Wf, Wb, gate):
        L = np.zeros((128, 128), f32)
        Wgf = np.asarray(Wf, f32)[gate * H:(gate + 1) * H, :]  # [32, 32]
        Wgb = np.asarray(Wb, f32)[gate * H:(gate + 1) * H, :]
        L[0:32, 0:32] = Wgf.T
        L[32:64, 32:64] = Wgf.T
        L[64:96, 64:96] = Wgb.T
        L[96:128, 96:128] = Wgb.T
        return L

    w = {
        "ihR_f": ih_tile(Wih_f, 0, False), "ihR_b": ih_tile(Wih_b, 0, True),
        "ihZ_f": ih_tile(Wih_f, 1, False), "ihZ_b": ih_tile(Wih_b, 1, True),
        "ihN_f": ih_tile(Wih_f, 2, False), "ihN_b": ih_tile(Wih_b, 2, True),
        "hhR": hh_tile(Whh_f, Whh_b, 0),
        "hhZ": hh_tile(Whh_f, Whh_b, 1),
        "hhN": hh_tile(Whh_f, Whh_b, 2),
    }

    def bias_vec(vf, vb):
        v = np.concatenate([np.tile(np.asarray(vf, f32), 2), np.tile(np.asarray(vb, f32), 2)])
        return np.ascontiguousarray(v.reshape(128, 1))

    bih_f, bhh_f = np.asarray(bih_f, f32), np.asarray(bhh_f, f32)
    bih_b, bhh_b = np.asarray(bih_b, f32), np.asarray(bhh_b, f32)
    b = {
        "biasR": bias_vec(bih_f[0:H] + bhh_f[0:H], bih_b[0:H] + bhh_b[0:H]),
        "biasZ": bias_vec(bih_f[H:2 * H] + bhh_f[H:2 * H], bih_b[H:2 * H] + bhh_b[H:2 * H]),
        "biasN": bias_vec(bih_f[2 * H:], bih_b[2 * H:]),
        "bhhN": bias_vec(bhh_f[2 * H:], bhh_b[2 * H:]),
    }

    in_maps = []
    for core in range(NCORES):
        xc = x_flat[core * WPC:(core + 1) * WPC]  # [2048, 16]
        # idx[16*cc + p, t, s16] = xc[g(cc)*1024 + s16*16 + p, t], g(cc)=cc//4
        xw = xc.reshape(2, G // 16, 16, T)  # [g, s16, p, t]
        arr = np.transpose(xw, (0, 2, 3, 1))  # [g, p, t, s16]
        idx = np.ascontiguousarray(
            np.stack([arr[cc // 4] for cc in range(8)]).reshape(128, T, G // 16)
        )
        m = {"idx": idx, "embT": embT2}
        for k, v in w.items():
            m[k] = v
        for k, v in b.items():
            m[k] = v
        in_maps.append(m)
    return in_maps


def _install_ntff_hook():
    """Register the axon NTFF profiling hook (the image's antenv lacks
    axon_hooks, so run_bass_kernel_spmd's trace path can't find it)."""
    import types
    import antenv

    if "antenv.axon_hooks" in sys.modules:
        return
    mod = types.ModuleType("antenv.axon_hooks")
    _h = {"hook": None}
    mod.set_axon_ntff_profile_hook = lambda h: _h.update(hook=h)
    mod.get_axon_ntff_profile_hook = lambda: _h["hook"]
    sys.modules["antenv.axon_hooks"] = mod
    antenv.axon_hooks = mod
    try:
        from trn_agent_boot.trn_boot import _ntff_profile_via_ctypes

        hook = _ntff_profile_via_ctypes("/opt/axon/libaxon_pjrt.so")
        if hook is not None:
            mod.set_axon_ntff_profile_hook(hook)
    except Exception as e:  # profiling is best-effort
        print("ntff hook install failed:", e)
    # artifact upload needs a bucket that doesn't exist in this sandbox
    import concourse.bass_utils as bu

    bu.upload_artifacts = lambda tmpdir: tmpdir


def kernel(x, emb, Wih_f, Whh_f, bih_f, bhh_f, Wih_b, Whh_b, bih_b, bhh_b):
    if "nc" not in _CACHE:
        _CACHE["nc"] = _build_program()
    nc = _CACHE["nc"]

    in_maps = _prep_inputs(
        x, emb, Wih_f, Whh_f, bih_f, bhh_f, Wih_b, Whh_b, bih_b, bhh_b
    )

    trace = bool(int(os.environ.get("CHAR_RNN_TRACE", "0")))
    if trace:
        _install_ntff_hook()
    res = run_bass_kernel_spmd(
        nc, in_maps, core_ids=list(range(NCORES)), trace=trace,
        trace_cores=[0] if trace else None,
    )
    _CACHE["last_results"] = res

    out = np.empty((16384, 2 * H), np.float32)
    for core in range(NCORES):
        o = res.results[core]["out"]  # [128, 1024]
        base = core * WPC
        out[base:base + G, 0:H] = o[0:32].T
        out[base:base + G, H:] = o[64:96].T
        out[base + G:base + WPC, 0:H] = o[32:64].T
        out[base + G:base + WPC, H:] = o[96:128].T
    return out.reshape(B, S, 2 * H)
